# revision 23
# baseline (speedup 1.0000x reference)
"""Trainium2 Bass kernel for nn_ModelNew_78847009620052 (dense_mlp).

Computes, for x [4096, 8192] and weight [8192, 8192]:
    out[b, 0] = 0.75 * sum_i x[b, i] * (sum_j weight[j, i])
(which equals 1.5 * sum(x @ W.T / 2, axis=1, keepdims=True)).

Sharding: column-shard the contraction dim IN=8192 into 8 chunks of 1024.
Core d receives x[:, d*1024:(d+1)*1024] and weight[:, d*1024:(d+1)*1024],
produces a partial [4096, 1]; host sums the 8 partials.

Per-core device algorithm (memory-bound: 48MB of input per core; the
per-pair HBM roofline is ~716 GB/s shared by 2 cores, so the target is
DMA-engine occupancy ~100% with w streamed strictly before x):

  All loads are issued on nc.sync (SP HWDGE) -> one FIFO descriptor ring,
  so the byte order on the wire is exactly the issue order. All w/x tiles
  share ONE tile-pool ring (tag "stream", 8 x 2MB bufs): each allocation's
  DMA can only be issued once the buffer's previous occupant was consumed,
  which (a) paces issue, (b) guarantees x DMAs enqueue after all w DMAs,
  so the weight stream gets full bandwidth until it completes, then x
  streams seamlessly while VectorE consumes it.

  Phase 1 (w, 32MB): tile 0 is DMA'd straight into a persistent 4-slot
  accumulator acc4 [128, 4, 1024] (4 x 0.5MB sub-DMAs, which also
  shortens the NEFF-preamble ramp); tiles 1..14 are 2MB tiles ([128,
  4096] via "(p t) c -> p (t c)", 16KB contiguous per-partition
  descriptors) folded into acc4 with ONE [128, 4096]-wide VectorE add
  each, then a 1MB + 2 x 0.5MB taper. Ring buffers are released by
  VectorE alone -- no cross-engine WAR feedback (per-tile TensorE
  matmuls created a standing ~14us pipeline lag in earlier designs).
  At the end: merge adds, two PE-warmup transposes reading taper tiles
  (never ring-reused, so they cannot backpressure the stream), and one
  matmul pair against a stationary 0.75-constant [128, 128] that
  reduces acc4 over partitions AND broadcasts the scaled column sums
  to all 128 PSUM partitions (0.75 folds the reference's /2 * 1.5)
  into psum_bc [128, 1024].

  Phase 2 (x, 16MB): 7 x 2MB tiles ([128, 4, 1024] via "(t p) c -> p t c"
  so tile slice t holds batch rows 512i+128t..+127 in partition order),
  then 1MB + 2 x 0.5MB (taper again). Each [128, 1024] slice is consumed
  by ONE VectorE scalar_tensor_tensor op reading the column sums straight
  from PSUM: out = (x * 1.0) * psum_bc, accum_out = row sums -> s_sbuf
  column n (n = global 128-row group index). Tile n covers batch rows
  128n..128n+127, so s_sbuf[p, n] = out[128n + p].

  Finish: s_sbuf [128, 32] is transposed on TensorE ([32, 128] in PSUM),
  copied to SBUF on ScalarE, and stored as one contiguous 16KB DMA.
"""

import numpy as np

B, IN, HID = 4096, 8192, 8192
N_CORES = 8
CHUNK = IN // N_CORES          # 1024 columns per core
SCALE = 1.5 / 2.0              # 0.75, folded into the ones matrix
P = 128                        # partitions
N_GROUPS = B // P              # 32 x row-groups per core

_compiled_nc = None


def _build_nc():
    import concourse.bass as bass
    import concourse.tile as tile
    from concourse import bacc, mybir
    from concourse.masks import make_identity

    f32 = mybir.dt.float32
    nc = bacc.Bacc(
        "TRN2",
        target_bir_lowering=False,
        debug=False,
        num_devices=N_CORES,
    )

    x_d = nc.dram_tensor("x", [B, CHUNK], f32, kind="ExternalInput")
    w_d = nc.dram_tensor("w", [HID, CHUNK], f32, kind="ExternalInput")
    out_d = nc.dram_tensor("out", [B, 1], f32, kind="ExternalOutput")

    with tile.TileContext(nc) as tc:
        with (
            tc.tile_pool(name="stream", bufs=10) as stream,
            tc.tile_pool(name="scratch", bufs=2) as scratch,
            tc.tile_pool(name="const", bufs=1) as const,
            tc.tile_pool(name="psum", bufs=1, space="PSUM") as psum_pool,
        ):
            ones = const.tile([P, P], f32)
            nc.vector.memset(ones[:], SCALE)
            identity = const.tile([P, P], f32)
            make_identity(nc, identity)
            s_sbuf = const.tile([P, N_GROUPS], f32)
            sT = const.tile([N_GROUPS, P], f32)
            sTB = const.tile([4, P], f32)

            psum_bc = psum_pool.tile([P, CHUNK], f32, tag="psum_bc")
            psum_tA = psum_pool.tile([28, P], f32, tag="psum_tA")
            psum_tB = psum_pool.tile([4, P], f32, tag="psum_tB")
            psum_junk = psum_pool.tile([P, P], f32, tag="psum_junk")

            def w_matmuls(src_ap, start, stop):
                for h in range(2):
                    nc.tensor.matmul(
                        psum_bc[:, h * 512 : (h + 1) * 512],
                        ones[:],
                        src_ap[:, h * 512 : (h + 1) * 512],
                        start=start,
                        stop=stop,
                    )

            # --- Phase 1: stream w, accumulate 0.75 * column sums ---
            # DVE-only accumulation: per 2MB tile, two [128, 2048] adds fold
            # the tile into a persistent accumulator. Ring buffers are
            # released by DVE alone -- no cross-engine feedback loop (in
            # v2-v5, ring WAR deps on PE matmuls locked the whole pipeline to
            # the DMA cadence with a standing ~14us phase lag). The
            # partition-reduce + broadcast matmuls run once, at the end.
            # 4-slot accumulator; tile 0 is DMA'd straight into it (its four
            # 0.5MB sub-DMAs double as the ramp-shortener: first bytes land
            # ~7us instead of a monolithic 2MB completing at ~14us).
            acc4 = const.tile([P, 4, CHUNK], f32)
            for t in range(4):
                nc.sync.dma_start(acc4[:, t, :], w_d[128 * t : 128 * (t + 1), :])

            # Tiles 1..14: ONE [128, 4096]-wide add per tile (4.7us incl the
            # DVE drain, under the fast-core DMA cadence of 4.85us) -- two
            # ops per tile made DVE the pacer in v6/v7.
            for j in range(1, 15):
                wl = stream.tile([P, 4, CHUNK], f32, tag="stream")
                nc.sync.dma_start(
                    wl[:],
                    w_d[j * 512 : (j + 1) * 512, :].rearrange(
                        "(p t) c -> p (t c)", p=P
                    ),
                )
                nc.vector.tensor_add(acc4[:], acc4[:], wl[:])

            # First merge level early (slots 2,3 are final after tile 14),
            # then fold the taper pieces into slots 0/1; only the last piece
            # add + final merge + matmul pair sit after the last w byte.
            nc.vector.tensor_add(
                acc4[:, 0:2, :], acc4[:, 0:2, :], acc4[:, 2:4, :]
            )
            pw = stream.tile([P, 4, CHUNK], f32, tag="stream")
            nc.sync.dma_start(
                pw[:, 0:2, :],
                w_d[7680:7936, :].rearrange("(p t) c -> p (t c)", p=P),
            )
            nc.vector.tensor_add(acc4[:, 0:2, :], acc4[:, 0:2, :], pw[:, 0:2, :])
            q1 = stream.tile([P, 4, CHUNK], f32, tag="stream")
            nc.sync.dma_start(q1[:, 0, :], w_d[7936:8064, :])
            nc.vector.tensor_add(acc4[:, 0, :], acc4[:, 0, :], q1[:, 0, :])
            q2 = stream.tile([P, 4, CHUNK], f32, tag="stream")
            nc.sync.dma_start(q2[:, 0, :], w_d[8064:8192, :])
            nc.vector.tensor_add(acc4[:, 1, :], acc4[:, 1, :], q2[:, 0, :])
            # PE warmups read only taper tiles (never ring-reused, so they
            # cannot backpressure the stream -- the v7 dummies did).
            nc.tensor.transpose(psum_junk[:], pw[:, 0, 0:P], identity[:])
            nc.tensor.transpose(psum_junk[:], q1[:, 0, 0:P], identity[:])
            nc.vector.tensor_add(acc4[:, 0, :], acc4[:, 0, :], acc4[:, 1, :])
            w_matmuls(acc4[:, 0, :], start=True, stop=True)

            # --- Phase 2: stream x, fused multiply+row-sum on VectorE ---
            def x_op(xl, t, n):
                scr = scratch.tile([P, CHUNK], f32, tag="scr")
                nc.vector.scalar_tensor_tensor(
                    out=scr[:],
                    in0=xl[:, t, :],
                    scalar=1.0,
                    in1=psum_bc[:],
                    op0=mybir.AluOpType.mult,
                    op1=mybir.AluOpType.mult,
                    accum_out=s_sbuf[:, n : n + 1],
                )

            # 1MB piece first (rows 3584..3839, n = 28, 29): it lands before
            # psum_bc closes, so the fused-op chain starts as early as psum_bc
            # allows instead of waiting for a full 2MB tile.
            xl = stream.tile([P, 4, CHUNK], f32, tag="stream")
            nc.sync.dma_start(
                xl[:, 0:2, :],
                x_d[3584:3840, :].rearrange("(t p) c -> p t c", p=P),
            )
            x_op(xl, 0, 28)
            x_op(xl, 1, 29)

            for i in range(7):  # 2MB tiles, rows 512i .. 512i+511
                xl = stream.tile([P, 4, CHUNK], f32, tag="stream")
                nc.sync.dma_start(
                    xl[:],
                    x_d[i * 512 : (i + 1) * 512, :].rearrange(
                        "(t p) c -> p t c", p=P
                    ),
                )
                for t in range(4):
                    x_op(xl, t, 4 * i + t)

            # two 0.5MB pieces (n = 30, 31)
            for k, (r0, r1) in enumerate([(3840, 3968), (3968, 4096)]):
                xl = stream.tile([P, 4, CHUNK], f32, tag="stream")
                nc.sync.dma_start(xl[:, 0, :], x_d[r0:r1, :])
                x_op(xl, 0, 30 + k)

            # --- Finish: transpose s to [32, 128] and store contiguously.
            # Piece A (rows 0..3583) finalizes while the x tail streams; the
            # stores issue on the ACT HWDGE ring so they bypass the SP ring's
            # FIFO (whose tail is still draining x bytes).
            nc.tensor.transpose(psum_tA[:], s_sbuf[:, 0:28], identity[:])
            nc.scalar.copy(sT[0:28, :], psum_tA[:])
            nc.scalar.dma_start(
                out_d[0:3584].rearrange("(n p) o -> n (p o)", p=P), sT[0:28, :]
            )
            nc.tensor.transpose(psum_tB[:], s_sbuf[:, 28:32], identity[:])
            nc.scalar.copy(sTB[:], psum_tB[:])
            nc.scalar.dma_start(
                out_d[3584:4096].rearrange("(n p) o -> n (p o)", p=P), sTB[:]
            )

    nc.compile()
    return nc


def _get_nc():
    global _compiled_nc
    if _compiled_nc is None:
        _compiled_nc = _build_nc()
    return _compiled_nc


def kernel(x: np.ndarray, weight: np.ndarray) -> np.ndarray:
    from concourse.bass_utils import run_bass_kernel_spmd

    x = np.asarray(x, dtype=np.float32)
    weight = np.asarray(weight, dtype=np.float32)
    assert x.shape == (B, IN) and weight.shape == (HID, IN)

    nc = _get_nc()
    in_maps = [
        {
            "x": np.ascontiguousarray(x[:, d * CHUNK : (d + 1) * CHUNK]),
            "w": np.ascontiguousarray(weight[:, d * CHUNK : (d + 1) * CHUNK]),
        }
        for d in range(N_CORES)
    ]
    res = run_bass_kernel_spmd(nc, in_maps, core_ids=list(range(N_CORES)))
    acc = np.zeros((B, 1), dtype=np.float64)
    for d in range(N_CORES):
        acc += res.results[d]["out"].astype(np.float64)
    return acc.astype(np.float32)
